# revision 7
# baseline (speedup 1.0000x reference)
"""Trainium2 Bass kernel for nn_Encoder (6-layer parallel-branch transformer encoder).

Sharding: sequence-split data-parallel over 8 cores. Core c owns the 512 tokens
[512*(c%2), 512*(c%2+1)) of batch element c//2. Per layer, core pairs (2b, 2b+1)
AllGather their K/V so each core attends over the full 1024-token sequence.
The attention mask is all-ones (verified at runtime; numpy fallback otherwise),
so key order is irrelevant and rank-indexed K/V blocks need no parity handling.

Layout: activations are kept feature-major ([features -> partitions, tokens ->
free]) end to end, which makes every projection a natural PE matmul with the
weight stationary and requires no on-device transposes. LayerNorm reductions
(over the partition axis) are done with ones-vector matmuls on the PE; softmax
denominators come for free from a ones-row appended to V in the AV matmul.

Matmuls run in float32r (full PE speed at N=512, ~2e-4 relative error/op).
"""

import sys

if "/opt/trn_rl_repo" not in sys.path:
    sys.path.insert(0, "/opt/trn_rl_repo")

import numpy as np

import concourse.bass as bass
import concourse.mybir as mybir
import concourse.tile as tile
from concourse import bacc
from concourse.bass_utils import run_bass_kernel_spmd

F32 = mybir.dt.float32
F32R = mybir.dt.float32r
AF = mybir.ActivationFunctionType
OP = mybir.AluOpType

P = 128          # partitions
B, S, DM, DFF = 4, 1024, 1024, 4096
H, DH = 16, 64   # heads, head dim
L = 6            # layers
T = 512          # tokens per core
FC = DM // P     # feature chunks (8)
HC = DFF // P    # ff chunks (32)
NCORES = 8
EPS = 1e-5
KV_K = DM * T          # floats in the kT region of the kv buffer
KV_V = T * DM          # floats in the v region
KV_TOT = KV_K + KV_V

_CACHED_NC = None


def _ln(nc, pools, xin, gcol, bcol, xn_out, ones):
    """LayerNorm over the feature (partition) axis, feature-major layout.

    xin: [128, FC, T] f32r; gcol/bcol: [128, FC] f32; xn_out: [128, FC, T].
    """
    psum_b, ptmp, pst, pbc = pools["psum_b"], pools["ptmp"], pools["pst"], pools["pbc"]

    sum_ps = psum_b.tile([P, T], F32, tag="pb")
    for fc in range(FC):
        nc.tensor.matmul(sum_ps[:1], ones[:], xin[:, fc, :],
                         start=(fc == 0), stop=(fc == FC - 1))
    sumsq_ps = psum_b.tile([P, T], F32, tag="pb")
    for fc in range(FC):
        sq = ptmp.tile([P, T], F32R, tag="sqr")
        nc.scalar.activation(sq[:], xin[:, fc, :], AF.Square)
        nc.tensor.matmul(sumsq_ps[:1], ones[:], sq[:],
                         start=(fc == 0), stop=(fc == FC - 1))

    m_sb = pst.tile([1, T], F32, tag="st")
    nc.vector.tensor_scalar_mul(m_sb[:], sum_ps[:1, :], 1.0 / DM)
    var = pst.tile([1, T], F32, tag="st")
    nc.vector.tensor_scalar_mul(var[:], sumsq_ps[:1, :], 1.0 / DM)
    mm = pst.tile([1, T], F32, tag="st")
    nc.vector.tensor_tensor(mm[:], m_sb[:], m_sb[:], OP.mult)
    nc.vector.tensor_tensor(var[:], var[:], mm[:], OP.subtract)
    nc.vector.tensor_scalar_add(var[:], var[:], EPS)
    rinv = pst.tile([1, T], F32, tag="st")
    nc.vector.reciprocal(rinv[:], var[:])
    r_sb = pst.tile([1, T], F32, tag="st")
    nc.scalar.activation(r_sb[:], rinv[:], AF.Sqrt)

    m_bt = pbc.tile([P, T], F32, tag="bc")
    r_bt = pbc.tile([P, T], F32, tag="bc")
    nc.gpsimd.partition_broadcast(m_bt[:], m_sb[:1, :])
    nc.gpsimd.partition_broadcast(r_bt[:], r_sb[:1, :])

    for fc in range(FC):
        t1 = ptmp.tile([P, T], F32, tag="t1")
        nc.vector.tensor_tensor(t1[:], xin[:, fc, :], m_bt[:], OP.subtract)
        nc.vector.tensor_tensor(t1[:], t1[:], r_bt[:], OP.mult)
        nc.scalar.activation(xn_out[:, fc, :], t1[:], AF.Identity,
                             bias=bcol[:, fc:fc + 1], scale=gcol[:, fc:fc + 1])


def _load_col(nc, pool, dram_vec, width, tag):
    """Load a [width*128] DRAM vector as a [128, width] per-partition column tile."""
    t = pool.tile([P, width], F32, tag=tag)
    nc.sync.dma_start(t[:], dram_vec.rearrange("(c p) -> p c", p=P))
    return t


def _build_program():
    nc = bacc.Bacc(None, target_bir_lowering=False, debug=False)

    xT = nc.dram_tensor("xT", [DM, T], F32R, kind="ExternalInput")
    wq = nc.dram_tensor("wq", [L, DM, DM], F32R, kind="ExternalInput")
    wk = nc.dram_tensor("wk", [L, DM, DM], F32R, kind="ExternalInput")
    wv = nc.dram_tensor("wv", [L, DM, DM], F32R, kind="ExternalInput")
    wo = nc.dram_tensor("wo", [L, DM, DM], F32R, kind="ExternalInput")
    w1 = nc.dram_tensor("w1", [L, DM, DFF], F32R, kind="ExternalInput")
    w2 = nc.dram_tensor("w2", [L, DFF, DM], F32R, kind="ExternalInput")
    bq = nc.dram_tensor("bq", [L, DM], F32, kind="ExternalInput")
    bk = nc.dram_tensor("bk", [L, DM], F32, kind="ExternalInput")
    bv = nc.dram_tensor("bv", [L, DM], F32, kind="ExternalInput")
    bo = nc.dram_tensor("bo", [L, DM], F32, kind="ExternalInput")
    b1 = nc.dram_tensor("b1", [L, DFF], F32, kind="ExternalInput")
    b2 = nc.dram_tensor("b2", [L, DM], F32, kind="ExternalInput")
    ln1g = nc.dram_tensor("ln1g", [L, DM], F32, kind="ExternalInput")
    ln1b = nc.dram_tensor("ln1b", [L, DM], F32, kind="ExternalInput")
    ln2g = nc.dram_tensor("ln2g", [L, DM], F32, kind="ExternalInput")
    ln2b = nc.dram_tensor("ln2b", [L, DM], F32, kind="ExternalInput")
    lnfg = nc.dram_tensor("lnfg", [DM], F32, kind="ExternalInput")
    lnfb = nc.dram_tensor("lnfb", [DM], F32, kind="ExternalInput")
    yT = nc.dram_tensor("yT", [DM, T], F32, kind="ExternalOutput")

    kv_send = [nc.dram_tensor(f"kv_send_{i}", [KV_TOT], F32) for i in range(L)]
    kv_recv = [nc.dram_tensor(f"kv_recv_{i}", [2, KV_TOT], F32) for i in range(L)]
    groups = [[0, 1], [2, 3], [4, 5], [6, 7]]

    from contextlib import ExitStack

    with tile.TileContext(nc) as tc:
        with ExitStack() as stack:
            ent = stack.enter_context
            px = ent(tc.tile_pool(name="px", bufs=1))
            pxn = ent(tc.tile_pool(name="pxn", bufs=2))
            pq = ent(tc.tile_pool(name="pq", bufs=1))
            pctx = ent(tc.tile_pool(name="pctx", bufs=1))
            pfacc = ent(tc.tile_pool(name="pfacc", bufs=1))
            pkv = ent(tc.tile_pool(name="pkv", bufs=3))
            pw5 = ent(tc.tile_pool(name="pw5", bufs=5))
            pw10 = ent(tc.tile_pool(name="pw10", bufs=5))
            pkhp = ent(tc.tile_pool(name="pkhp", bufs=2))
            pvhp = ent(tc.tile_pool(name="pvhp", bufs=2))
            pexp = ent(tc.tile_pool(name="pexp", bufs=3))
            ph = ent(tc.tile_pool(name="ph", bufs=5))
            pcol = ent(tc.tile_pool(name="pcol", bufs=10))
            pst = ent(tc.tile_pool(name="pst", bufs=6))
            pbc = ent(tc.tile_pool(name="pbc", bufs=2))
            ptmp = ent(tc.tile_pool(name="ptmp", bufs=2))
            pones = ent(tc.tile_pool(name="pones", bufs=1))
            psum_a = ent(tc.tile_pool(name="psum_a", bufs=4, space="PSUM"))
            psum_b = ent(tc.tile_pool(name="psum_b", bufs=3, space="PSUM"))

            pools = {"psum_b": psum_b, "ptmp": ptmp, "pst": pst, "pbc": pbc}

            ones_f = pones.tile([P, 1], F32)
            nc.vector.memset(ones_f[:], 1.0)
            ones = pones.tile([P, 1], F32R)
            nc.vector.tensor_copy(ones[:], ones_f[:])

            x_sb = px.tile([P, FC, T], F32R)
            nc.sync.dma_start(x_sb[:], xT.rearrange("(c p) t -> p c t", p=P))

            for i in range(L):
                # ---- per-layer constant columns ----
                l1g = _load_col(nc, pcol, ln1g[i], FC, "c8")
                l1b = _load_col(nc, pcol, ln1b[i], FC, "c8")
                l2g = _load_col(nc, pcol, ln2g[i], FC, "c8")
                l2b = _load_col(nc, pcol, ln2b[i], FC, "c8")
                bqc = _load_col(nc, pcol, bq[i], FC, "c8")
                bkc = _load_col(nc, pcol, bk[i], FC, "c8")
                boc = _load_col(nc, pcol, bo[i], FC, "c8")
                b2c = _load_col(nc, pcol, b2[i], FC, "c8")
                b1c = _load_col(nc, pcol, b1[i], HC, "c32")

                # ---- LN1 (attention branch input) ----
                xn1 = pxn.tile([P, FC, T], F32R, tag="xn")
                _ln(nc, pools, x_sb, l1g, l1b, xn1, ones)

                # ---- Q projection (own tokens), feature-major out ----
                qT = pq.tile([P, FC, T], F32R)
                for mcg in range(2):
                    ps = [psum_a.tile([P, T], F32, tag="pa", name=f"pa{_j}") for _j in range(4)]
                    for fc in range(FC):
                        wt = pw5.tile([P, 512], F32R, tag="w5")
                        nc.sync.dma_start(
                            wt[:], wq[i, fc * P:(fc + 1) * P, mcg * 512:(mcg + 1) * 512])
                        for j in range(4):
                            nc.tensor.matmul(ps[j][:], wt[:, j * P:(j + 1) * P],
                                             xn1[:, fc, :],
                                             start=(fc == 0), stop=(fc == FC - 1))
                    for j in range(4):
                        mc = mcg * 4 + j
                        nc.vector.tensor_scalar_add(qT[:, mc, :], ps[j][:],
                                                    bqc[:, mc:mc + 1])

                # ---- K projection (own tokens) -> kv_send ----
                send_k = kv_send[i][0:KV_K].rearrange("(d t) -> d t", t=T)
                for mcg in range(2):
                    ps = [psum_a.tile([P, T], F32, tag="pa", name=f"pa{_j}") for _j in range(4)]
                    for fc in range(FC):
                        wt = pw5.tile([P, 512], F32R, tag="w5")
                        nc.sync.dma_start(
                            wt[:], wk[i, fc * P:(fc + 1) * P, mcg * 512:(mcg + 1) * 512])
                        for j in range(4):
                            nc.tensor.matmul(ps[j][:], wt[:, j * P:(j + 1) * P],
                                             xn1[:, fc, :],
                                             start=(fc == 0), stop=(fc == FC - 1))
                    for j in range(4):
                        mc = mcg * 4 + j
                        kvt = pkv.tile([P, T], F32, tag="kv")
                        nc.vector.tensor_scalar_add(kvt[:], ps[j][:], bkc[:, mc:mc + 1])
                        nc.sync.dma_start(send_k[mc * P:(mc + 1) * P, :], kvt[:])

                # ---- V projection (own tokens, token-major out) -> kv_send ----
                send_v = kv_send[i][KV_K:].rearrange("(tk d) -> tk d", d=DM)
                for dh2 in range(2):
                    bvrow = pst.tile([1, 512], F32, tag="bvr")
                    nc.sync.dma_start(bvrow[:], bv[i, dh2 * 512:(dh2 + 1) * 512][None, :])
                    bv_bt = pbc.tile([P, 512], F32, tag="bc")
                    nc.gpsimd.partition_broadcast(bv_bt[:], bvrow[:1, :])
                    ps = [psum_a.tile([P, 512], F32, tag="pa", name=f"pa{_j}") for _j in range(4)]
                    for fc in range(FC):
                        wt = pw5.tile([P, 512], F32R, tag="w5")
                        nc.sync.dma_start(
                            wt[:], wv[i, fc * P:(fc + 1) * P, dh2 * 512:(dh2 + 1) * 512])
                        for tc4 in range(4):
                            nc.tensor.matmul(ps[tc4][:],
                                             xn1[:, fc, tc4 * P:(tc4 + 1) * P], wt[:],
                                             start=(fc == 0), stop=(fc == FC - 1))
                    for tc4 in range(4):
                        kvt = pkv.tile([P, 512], F32, tag="kv")
                        nc.vector.tensor_tensor(kvt[:], ps[tc4][:], bv_bt[:], OP.add)
                        nc.sync.dma_start(
                            send_v[tc4 * P:(tc4 + 1) * P, dh2 * 512:(dh2 + 1) * 512],
                            kvt[:])

                # ---- AllGather K/V within the core pair ----
                nc.gpsimd.collective_compute(
                    "AllGather", OP.bypass,
                    ins=[kv_send[i][:]], outs=[kv_recv[i][:]],
                    replica_groups=groups)

                # ---- LN2 + FFN branch (independent of attention) ----
                xn2 = pxn.tile([P, FC, T], F32R, tag="xn")
                _ln(nc, pools, x_sb, l2g, l2b, xn2, ones)

                ffacc = pfacc.tile([P, FC, T], F32)
                for g in range(8):
                    # h chunks hc = 4g..4g+3
                    ps = [psum_a.tile([P, T], F32, tag="pa", name=f"pa{_j}") for _j in range(4)]
                    for fc in range(FC):
                        wt = pw5.tile([P, 512], F32R, tag="w5")
                        nc.sync.dma_start(
                            wt[:], w1[i, fc * P:(fc + 1) * P, g * 512:(g + 1) * 512])
                        for j in range(4):
                            nc.tensor.matmul(ps[j][:], wt[:, j * P:(j + 1) * P],
                                             xn2[:, fc, :],
                                             start=(fc == 0), stop=(fc == FC - 1))
                    hts = []
                    for j in range(4):
                        hc = g * 4 + j
                        ht = ph.tile([P, T], F32R, tag="h")
                        nc.scalar.activation(ht[:], ps[j][:], AF.Relu,
                                             bias=b1c[:, hc:hc + 1])
                        hts.append(ht)
                    w2ts = []
                    for j in range(4):
                        hc = g * 4 + j
                        w2t = pw10.tile([P, DM], F32R, tag="w10")
                        nc.sync.dma_start(w2t[:], w2[i, hc * P:(hc + 1) * P, :])
                        w2ts.append(w2t)
                    for mc in range(FC):
                        wps = psum_b.tile([P, T], F32, tag="pb")
                        for j in range(4):
                            nc.tensor.matmul(wps[:], w2ts[j][:, mc * P:(mc + 1) * P],
                                             hts[j][:],
                                             start=(j == 0), stop=(j == 3))
                        if g == 0:
                            nc.vector.tensor_scalar_add(ffacc[:, mc, :], wps[:],
                                                        b2c[:, mc:mc + 1])
                        else:
                            nc.vector.tensor_tensor(ffacc[:, mc, :], ffacc[:, mc, :],
                                                    wps[:], OP.add)

                # ---- Attention over gathered K/V ----
                recv_k = [kv_recv[i][r, 0:KV_K].rearrange("(d t) -> d t", t=T)
                          for r in range(2)]
                recv_v = [kv_recv[i][r, KV_K:].rearrange("(tk d) -> tk d", d=DM)
                          for r in range(2)]
                ctxT = pctx.tile([P, FC, T], F32R)
                for hp in range(FC):
                    kT_hp = pkhp.tile([P, 2 * T], F32R, tag="khp")
                    for r in range(2):
                        nc.sync.dma_start(
                            kT_hp[:, r * T:(r + 1) * T],
                            recv_k[r][hp * P:(hp + 1) * P, :].bitcast(F32R))
                    v_hp = pvhp.tile([P, 8, 2, 65], F32R, tag="vhp")
                    for r in range(2):
                        for h2 in range(2):
                            src = recv_v[r][:, hp * P + h2 * DH: hp * P + (h2 + 1) * DH]
                            src = src.rearrange("(tc p) d -> p tc d", p=P)
                            nc.sync.dma_start(v_hp[:, r * 4:(r + 1) * 4, h2, 0:64],
                                              src.bitcast(F32R))
                    nc.vector.tensor_copy(
                        v_hp[:, :, :, 64:65],
                        ones[:, :1][:, None, None, :].to_broadcast([P, 8, 2, 1]))

                    for h2 in range(2):
                        av_ps = psum_b.tile([P, T], F32, tag="pb")
                        for kc in range(8):
                            sc_ps = psum_a.tile([P, T], F32, tag="pa")
                            nc.tensor.matmul(
                                sc_ps[:],
                                kT_hp[h2 * DH:(h2 + 1) * DH, kc * P:(kc + 1) * P],
                                qT[h2 * DH:(h2 + 1) * DH, hp, :],
                                start=True, stop=True)
                            et = pexp.tile([P, T], F32R, tag="e")
                            nc.scalar.activation(et[:], sc_ps[:], AF.Exp, scale=0.125)
                            nc.tensor.matmul(av_ps[0:65], v_hp[:, kc, h2, :], et[:],
                                             start=(kc == 0), stop=(kc == 7))
                        recip = pst.tile([1, T], F32, tag="st")
                        nc.vector.reciprocal(recip[:], av_ps[64:65, :])
                        rb = pbc.tile([64, T], F32, tag="rb2")
                        nc.gpsimd.partition_broadcast(rb[:], recip[:1, :])
                        nc.vector.tensor_tensor(
                            ctxT[h2 * DH:(h2 + 1) * DH, hp, :],
                            av_ps[0:DH, :], rb[:], OP.mult)

                # ---- Output projection + residuals ----
                for mcg in range(2):
                    ps = [psum_a.tile([P, T], F32, tag="pa", name=f"pa{_j}") for _j in range(4)]
                    for fc in range(FC):
                        wt = pw5.tile([P, 512], F32R, tag="w5")
                        nc.sync.dma_start(
                            wt[:], wo[i, fc * P:(fc + 1) * P, mcg * 512:(mcg + 1) * 512])
                        for j in range(4):
                            nc.tensor.matmul(ps[j][:], wt[:, j * P:(j + 1) * P],
                                             ctxT[:, fc, :],
                                             start=(fc == 0), stop=(fc == FC - 1))
                    for j in range(4):
                        mc = mcg * 4 + j
                        t1 = ptmp.tile([P, T], F32, tag="t1")
                        nc.vector.tensor_scalar_add(t1[:], ps[j][:], boc[:, mc:mc + 1])
                        nc.vector.tensor_tensor(x_sb[:, mc, :], x_sb[:, mc, :],
                                                t1[:], OP.add)
                for mc in range(FC):
                    nc.vector.tensor_tensor(x_sb[:, mc, :], x_sb[:, mc, :],
                                            ffacc[:, mc, :], OP.add)

            # ---- final LN -> output ----
            lfg = _load_col(nc, pcol, lnfg, FC, "c8")
            lfb = _load_col(nc, pcol, lnfb, FC, "c8")
            yln = pxn.tile([P, FC, T], F32R, tag="xn")
            _ln(nc, pools, x_sb, lfg, lfb, yln, ones)
            yT_v = yT.rearrange("(c p) t -> p c t", p=P)
            for fc in range(FC):
                nc.sync.dma_start(yT_v[:, fc, :], yln[:, fc, :].bitcast(F32))

    nc.compile()
    return nc


def _get_program():
    global _CACHED_NC
    if _CACHED_NC is None:
        _CACHED_NC = _build_program()
    return _CACHED_NC


def _numpy_fallback(x, mask, wq, bq, wk, bk, wv, bv, wo, bo, w1, b1, w2, b2,
                    ln1g, ln1b, ln2g, ln2b, lnfg, lnfb):
    def ln(t, g, b):
        m = t.mean(-1, keepdims=True)
        v = ((t - m) ** 2).mean(-1, keepdims=True)
        return (t - m) / np.sqrt(v + EPS) * g + b

    x = x.astype(np.float32).copy()
    Bn, Sn, Dm = x.shape
    scale = 1.0 / np.sqrt(np.float32(DH))
    maskb = (mask == 0)[:, None]
    for i in range(L):
        xn1 = ln(x, ln1g[i], ln1b[i])
        xn2 = ln(x, ln2g[i], ln2b[i])

        def heads(t):
            return t.reshape(Bn, Sn, H, DH).transpose(0, 2, 1, 3)

        q = heads(xn1 @ wq[i] + bq[i])
        k = heads(xn1 @ wk[i] + bk[i])
        v = heads(xn1 @ wv[i] + bv[i])
        sc = np.einsum("bhqd,bhkd->bhqk", q, k) * scale
        sc = np.where(maskb, np.float32(-1e9), sc)
        sc = sc - sc.max(-1, keepdims=True)
        e = np.exp(sc)
        attn = e / e.sum(-1, keepdims=True)
        ctx = np.einsum("bhqk,bhkd->bhqd", attn, v)
        ctx = ctx.transpose(0, 2, 1, 3).reshape(Bn, Sn, Dm)
        x = x + (ctx @ wo[i] + bo[i])
        h = np.maximum(xn2 @ w1[i] + b1[i], 0.0)
        x = x + (h @ w2[i] + b2[i])
    return ln(x, lnfg, lnfb)


def kernel(**inputs):
    x = np.asarray(inputs["x"], dtype=np.float32)
    mask = np.asarray(inputs["mask"])
    if not (mask == 1).all():
        return _numpy_fallback(**{k: np.asarray(v) for k, v in inputs.items()})

    nc = _get_program()

    shared = {}
    for name in ("wq", "wk", "wv", "wo", "w1", "w2", "bq", "bk", "bv", "bo",
                 "b1", "b2", "ln1g", "ln1b", "ln2g", "ln2b", "lnfg", "lnfb"):
        shared[name] = np.ascontiguousarray(np.asarray(inputs[name], dtype=np.float32))

    in_maps = []
    for c in range(NCORES):
        b, s = c // 2, c % 2
        xT = np.ascontiguousarray(x[b, s * T:(s + 1) * T, :].T)
        in_maps.append({"xT": xT, **shared})

    res = run_bass_kernel_spmd(nc, in_maps, list(range(NCORES)))

    out = np.empty((B, S, DM), dtype=np.float32)
    for c in range(NCORES):
        b, s = c // 2, c % 2
        out[b, s * T:(s + 1) * T, :] = res.results[c]["yT"].T
    return out


# revision 12
# speedup vs baseline: 1.0540x; 1.0540x over previous
"""Trainium2 Bass kernel for nn_Encoder (6-layer parallel-branch transformer encoder).

Sharding: sequence-split data-parallel over 8 cores. Core c owns the 512 tokens
[512*(c%2), 512*(c%2+1)) of batch element c//2. Per layer, core pairs (2b, 2b+1)
AllGather their K/V so each core attends over the full 1024-token sequence.
The attention mask is all-ones (verified at runtime; numpy fallback otherwise),
so key order is irrelevant and rank-indexed K/V blocks need no parity handling.

Layout: activations are kept feature-major ([features -> partitions, tokens ->
free]) end to end, which makes every projection a natural PE matmul with the
weight stationary and requires no on-device transposes. LayerNorm reductions
(over the partition axis) are done with ones-vector matmuls on the PE; softmax
denominators come for free from a ones-row appended to V in the AV matmul.

Matmuls run in float32r (full PE speed at N=512, ~2e-4 relative error/op).
"""

import sys

if "/opt/trn_rl_repo" not in sys.path:
    sys.path.insert(0, "/opt/trn_rl_repo")

import numpy as np

import concourse.bass as bass
import concourse.mybir as mybir
import concourse.tile as tile
from concourse import bacc
from concourse.bass_utils import run_bass_kernel_spmd

F32 = mybir.dt.float32
F32R = mybir.dt.float32r
AF = mybir.ActivationFunctionType
OP = mybir.AluOpType

P = 128          # partitions
B, S, DM, DFF = 4, 1024, 1024, 4096
H, DH = 16, 64   # heads, head dim
L = 6            # layers
T = 512          # tokens per core
FC = DM // P     # feature chunks (8)
HC = DFF // P    # ff chunks (32)
NCORES = 8
EPS = 1e-5
KV_K = DM * T          # floats in the kT region of the kv buffer
KV_V = T * DM          # floats in the v region
KV_TOT = KV_K + KV_V

_CACHED_NC = None


def _ln(nc, pools, xin, gcol, bcol, xn_out, ones):
    """LayerNorm over the feature (partition) axis, feature-major layout.

    xin: [128, FC, T] f32r; gcol/bcol: [128, FC] f32; xn_out: [128, FC, T].
    """
    psum_b, ptmp, pst, pbc = pools["psum_b"], pools["ptmp"], pools["pst"], pools["pbc"]

    sum_ps = psum_b.tile([P, T], F32, tag="pb")
    for fc in range(FC):
        nc.tensor.matmul(sum_ps[:1], ones[:], xin[:, fc, :],
                         start=(fc == 0), stop=(fc == FC - 1))
    sumsq_ps = psum_b.tile([P, T], F32, tag="pb")
    for fc in range(FC):
        sq = ptmp.tile([P, T], F32R, tag="sqr")
        nc.scalar.activation(sq[:], xin[:, fc, :], AF.Square)
        nc.tensor.matmul(sumsq_ps[:1], ones[:], sq[:],
                         start=(fc == 0), stop=(fc == FC - 1))

    m_sb = pst.tile([1, T], F32, tag="st")
    nc.vector.tensor_scalar_mul(m_sb[:], sum_ps[:1, :], 1.0 / DM)
    var = pst.tile([1, T], F32, tag="st")
    nc.vector.tensor_scalar_mul(var[:], sumsq_ps[:1, :], 1.0 / DM)
    mm = pst.tile([1, T], F32, tag="st")
    nc.vector.tensor_tensor(mm[:], m_sb[:], m_sb[:], OP.mult)
    nc.vector.tensor_tensor(var[:], var[:], mm[:], OP.subtract)
    nc.vector.tensor_scalar_add(var[:], var[:], EPS)
    rinv = pst.tile([1, T], F32, tag="st")
    nc.vector.reciprocal_approx_fast(rinv[:], var[:])
    r_sb = pst.tile([1, T], F32, tag="st")
    nc.scalar.activation(r_sb[:], rinv[:], AF.Sqrt)

    m_bt = pbc.tile([P, T], F32, tag="bc")
    r_bt = pbc.tile([P, T], F32, tag="bc")
    nc.gpsimd.partition_broadcast(m_bt[:], m_sb[:1, :])
    nc.gpsimd.partition_broadcast(r_bt[:], r_sb[:1, :])

    for fc in range(FC):
        t1 = ptmp.tile([P, T], F32, tag="t1")
        nc.vector.tensor_tensor(t1[:], xin[:, fc, :], m_bt[:], OP.subtract)
        nc.vector.tensor_tensor(t1[:], t1[:], r_bt[:], OP.mult)
        nc.scalar.activation(xn_out[:, fc, :], t1[:], AF.Identity,
                             bias=bcol[:, fc:fc + 1], scale=gcol[:, fc:fc + 1])


def _load_col(nc, pool, dram_vec, width, tag):
    """Load a [width*128] DRAM vector as a [128, width] per-partition column tile."""
    t = pool.tile([P, width], F32, tag=tag)
    nc.sync.dma_start(t[:], dram_vec.rearrange("(c p) -> p c", p=P))
    return t


def _build_program():
    nc = bacc.Bacc(None, target_bir_lowering=False, debug=False)

    xT = nc.dram_tensor("xT", [DM, T], F32R, kind="ExternalInput")
    wq = nc.dram_tensor("wq", [L, DM, DM], F32R, kind="ExternalInput")
    wk = nc.dram_tensor("wk", [L, DM, DM], F32R, kind="ExternalInput")
    wv = nc.dram_tensor("wv", [L, DM, DM], F32R, kind="ExternalInput")
    wo = nc.dram_tensor("wo", [L, DM, DM], F32R, kind="ExternalInput")
    w1 = nc.dram_tensor("w1", [L, DM, DFF], F32R, kind="ExternalInput")
    w2 = nc.dram_tensor("w2", [L, DFF, DM], F32R, kind="ExternalInput")
    bq = nc.dram_tensor("bq", [L, DM], F32, kind="ExternalInput")
    bk = nc.dram_tensor("bk", [L, DM], F32, kind="ExternalInput")
    bv = nc.dram_tensor("bv", [L, DM], F32, kind="ExternalInput")
    bo = nc.dram_tensor("bo", [L, DM], F32, kind="ExternalInput")
    b1 = nc.dram_tensor("b1", [L, DFF], F32, kind="ExternalInput")
    b2 = nc.dram_tensor("b2", [L, DM], F32, kind="ExternalInput")
    ln1g = nc.dram_tensor("ln1g", [L, DM], F32, kind="ExternalInput")
    ln1b = nc.dram_tensor("ln1b", [L, DM], F32, kind="ExternalInput")
    ln2g = nc.dram_tensor("ln2g", [L, DM], F32, kind="ExternalInput")
    ln2b = nc.dram_tensor("ln2b", [L, DM], F32, kind="ExternalInput")
    lnfg = nc.dram_tensor("lnfg", [DM], F32, kind="ExternalInput")
    lnfb = nc.dram_tensor("lnfb", [DM], F32, kind="ExternalInput")
    yT = nc.dram_tensor("yT", [DM, T], F32, kind="ExternalOutput")

    kv_send = [nc.dram_tensor(f"kv_send_{i}", [KV_TOT], F32) for i in range(L)]
    kv_recv = [nc.dram_tensor(f"kv_recv_{i}", [2, KV_TOT], F32) for i in range(L)]
    groups = [[0, 1], [2, 3], [4, 5], [6, 7]]

    from contextlib import ExitStack

    with tile.TileContext(nc) as tc:
        with ExitStack() as stack:
            ent = stack.enter_context
            px = ent(tc.tile_pool(name="px", bufs=1))
            pxn = ent(tc.tile_pool(name="pxn", bufs=2))
            pq = ent(tc.tile_pool(name="pq", bufs=1))
            pctx = ent(tc.tile_pool(name="pctx", bufs=1))
            pfacc = ent(tc.tile_pool(name="pfacc", bufs=1))
            pkv = ent(tc.tile_pool(name="pkv", bufs=3))
            pw5 = ent(tc.tile_pool(name="pw5", bufs=10))
            pw10 = ent(tc.tile_pool(name="pw10", bufs=5))
            pkhp = ent(tc.tile_pool(name="pkhp", bufs=2))
            pvhp = ent(tc.tile_pool(name="pvhp", bufs=2))
            pexp = ent(tc.tile_pool(name="pexp", bufs=4))
            ph = ent(tc.tile_pool(name="ph", bufs=5))
            pcol = ent(tc.tile_pool(name="pcol", bufs=10))
            pst = ent(tc.tile_pool(name="pst", bufs=5))
            pbv = ent(tc.tile_pool(name="pbv", bufs=1))
            pbc = ent(tc.tile_pool(name="pbc", bufs=2))
            ptmp = ent(tc.tile_pool(name="ptmp", bufs=2))
            pones = ent(tc.tile_pool(name="pones", bufs=1))
            psum_a = ent(tc.tile_pool(name="psum_a", bufs=4, space="PSUM"))
            psum_b = ent(tc.tile_pool(name="psum_b", bufs=3, space="PSUM"))

            pools = {"psum_b": psum_b, "ptmp": ptmp, "pst": pst, "pbc": pbc}

            ones_f = pones.tile([P, 1], F32)
            nc.vector.memset(ones_f[:], 1.0)
            ones = pones.tile([P, 1], F32R)
            nc.vector.tensor_copy(ones[:], ones_f[:])

            x_sb = px.tile([P, FC, T], F32R)
            nc.sync.dma_start(x_sb[:], xT.rearrange("(c p) t -> p c t", p=P))

            for i in range(L):
                # ---- per-layer constant columns ----
                l1g = _load_col(nc, pcol, ln1g[i], FC, "c8")
                l1b = _load_col(nc, pcol, ln1b[i], FC, "c8")
                l2g = _load_col(nc, pcol, ln2g[i], FC, "c8")
                l2b = _load_col(nc, pcol, ln2b[i], FC, "c8")
                bqc = _load_col(nc, pcol, bq[i], FC, "c8")
                bkc = _load_col(nc, pcol, bk[i], FC, "c8")
                boc = _load_col(nc, pcol, bo[i], FC, "c8")
                b2c = _load_col(nc, pcol, b2[i], FC, "c8")
                b1c = _load_col(nc, pcol, b1[i], HC, "c32")

                # ---- LN1 (attention branch input) ----
                xn1 = pxn.tile([P, FC, T], F32R, tag="xn")
                _ln(nc, pools, x_sb, l1g, l1b, xn1, ones)
                xn2 = pxn.tile([P, FC, T], F32R, tag="xn")
                _ln(nc, pools, x_sb, l2g, l2b, xn2, ones)

                # ---- Q projection (own tokens), feature-major out ----
                qT = pq.tile([P, FC, T], F32R)
                for mcg in range(2):
                    ps = [psum_a.tile([P, T], F32, tag="pa", name=f"pa{_j}") for _j in range(4)]
                    for fc in range(FC):
                        wt = pw5.tile([P, 512], F32R, tag="w5")
                        nc.sync.dma_start(
                            wt[:], wq[i, fc * P:(fc + 1) * P, mcg * 512:(mcg + 1) * 512])
                        for j in range(4):
                            nc.tensor.matmul(ps[j][:], wt[:, j * P:(j + 1) * P],
                                             xn1[:, fc, :],
                                             start=(fc == 0), stop=(fc == FC - 1))
                    for j in range(4):
                        mc = mcg * 4 + j
                        nc.vector.tensor_scalar_add(qT[:, mc, :], ps[j][:],
                                                    bqc[:, mc:mc + 1])

                # ---- K projection (own tokens) -> kv_send ----
                send_k = kv_send[i][0:KV_K].rearrange("(d t) -> d t", t=T)
                for mcg in range(2):
                    ps = [psum_a.tile([P, T], F32, tag="pa", name=f"pa{_j}") for _j in range(4)]
                    for fc in range(FC):
                        wt = pw5.tile([P, 512], F32R, tag="w5")
                        nc.sync.dma_start(
                            wt[:], wk[i, fc * P:(fc + 1) * P, mcg * 512:(mcg + 1) * 512])
                        for j in range(4):
                            nc.tensor.matmul(ps[j][:], wt[:, j * P:(j + 1) * P],
                                             xn1[:, fc, :],
                                             start=(fc == 0), stop=(fc == FC - 1))
                    for j in range(4):
                        mc = mcg * 4 + j
                        kvt = pkv.tile([P, T], F32, tag="kv")
                        nc.vector.tensor_scalar_add(kvt[:], ps[j][:], bkc[:, mc:mc + 1])
                        nc.sync.dma_start(send_k[mc * P:(mc + 1) * P, :], kvt[:])

                # ---- V projection (own tokens, token-major out) -> kv_send ----
                send_v = kv_send[i][KV_K:].rearrange("(tk d) -> tk d", d=DM)
                for dh2 in range(2):
                    bvrow = pbv.tile([1, 512], F32, tag="bvr")
                    nc.sync.dma_start(bvrow[:], bv[i, dh2 * 512:(dh2 + 1) * 512][None, :])
                    bv_bt = pbc.tile([P, 512], F32, tag="bc")
                    nc.gpsimd.partition_broadcast(bv_bt[:], bvrow[:1, :])
                    ps = [psum_a.tile([P, 512], F32, tag="pa", name=f"pa{_j}") for _j in range(4)]
                    for fc in range(FC):
                        wt = pw5.tile([P, 512], F32R, tag="w5")
                        nc.sync.dma_start(
                            wt[:], wv[i, fc * P:(fc + 1) * P, dh2 * 512:(dh2 + 1) * 512])
                        for tc4 in range(4):
                            nc.tensor.matmul(ps[tc4][:],
                                             xn1[:, fc, tc4 * P:(tc4 + 1) * P], wt[:],
                                             start=(fc == 0), stop=(fc == FC - 1))
                    for tc4 in range(4):
                        kvt = pkv.tile([P, 512], F32, tag="kv")
                        nc.vector.tensor_tensor(kvt[:], ps[tc4][:], bv_bt[:], OP.add)
                        nc.sync.dma_start(
                            send_v[tc4 * P:(tc4 + 1) * P, dh2 * 512:(dh2 + 1) * 512],
                            kvt[:])

                # ---- AllGather K/V within the core pair ----
                nc.gpsimd.collective_compute(
                    "AllGather", OP.bypass,
                    ins=[kv_send[i][:]], outs=[kv_recv[i][:]],
                    replica_groups=groups)

                # ---- FFN branch (xn2 computed up top) ----
                ffacc = pfacc.tile([P, FC, T], F32)
                for g in range(8):
                    # h chunks hc = 4g..4g+3
                    ps = [psum_a.tile([P, T], F32, tag="pa", name=f"pa{_j}") for _j in range(4)]
                    for fc in range(FC):
                        wt = pw5.tile([P, 512], F32R, tag="w5")
                        nc.sync.dma_start(
                            wt[:], w1[i, fc * P:(fc + 1) * P, g * 512:(g + 1) * 512])
                        for j in range(4):
                            nc.tensor.matmul(ps[j][:], wt[:, j * P:(j + 1) * P],
                                             xn2[:, fc, :],
                                             start=(fc == 0), stop=(fc == FC - 1))
                    hts = []
                    for j in range(4):
                        hc = g * 4 + j
                        ht = ph.tile([P, T], F32R, tag="h")
                        nc.scalar.activation(ht[:], ps[j][:], AF.Relu,
                                             bias=b1c[:, hc:hc + 1])
                        hts.append(ht)
                    w2ts = []
                    for j in range(4):
                        hc = g * 4 + j
                        w2t = pw10.tile([P, DM], F32R, tag="w10")
                        nc.sync.dma_start(w2t[:], w2[i, hc * P:(hc + 1) * P, :])
                        w2ts.append(w2t)
                    for mc in range(FC):
                        wps = psum_b.tile([P, T], F32, tag="pb")
                        for j in range(4):
                            nc.tensor.matmul(wps[:], w2ts[j][:, mc * P:(mc + 1) * P],
                                             hts[j][:],
                                             start=(j == 0), stop=(j == 3))
                        if g == 0:
                            nc.vector.tensor_scalar_add(ffacc[:, mc, :], wps[:],
                                                        b2c[:, mc:mc + 1])
                        else:
                            nc.vector.tensor_tensor(ffacc[:, mc, :], ffacc[:, mc, :],
                                                    wps[:], OP.add)

                # ---- Attention over gathered K/V ----
                recv_k = [kv_recv[i][r, 0:KV_K].rearrange("(d t) -> d t", t=T)
                          for r in range(2)]
                recv_v = [kv_recv[i][r, KV_K:].rearrange("(tk d) -> tk d", d=DM)
                          for r in range(2)]
                ctxT = pctx.tile([P, FC, T], F32R)
                for hp in range(FC):
                    kT_hp = pkhp.tile([P, 2 * T], F32R, tag="khp")
                    for r in range(2):
                        nc.sync.dma_start(
                            kT_hp[:, r * T:(r + 1) * T],
                            recv_k[r][hp * P:(hp + 1) * P, :].bitcast(F32R))
                    v_hp = pvhp.tile([P, 8, 2, 65], F32R, tag="vhp")
                    for r in range(2):
                        for h2 in range(2):
                            src = recv_v[r][:, hp * P + h2 * DH: hp * P + (h2 + 1) * DH]
                            src = src.rearrange("(tc p) d -> p tc d", p=P)
                            nc.sync.dma_start(v_hp[:, r * 4:(r + 1) * 4, h2, 0:64],
                                              src.bitcast(F32R))
                    nc.vector.tensor_copy(
                        v_hp[:, :, :, 64:65],
                        ones[:, :1][:, None, None, :].to_broadcast([P, 8, 2, 1]))

                    av_pair = [psum_b.tile([P, T], F32, tag="pb", name=f"av{_h}")
                               for _h in range(2)]
                    for kc in range(8):
                        for h2 in range(2):
                            sc_ps = psum_a.tile([P, T], F32, tag="pa")
                            nc.tensor.matmul(
                                sc_ps[:],
                                kT_hp[h2 * DH:(h2 + 1) * DH, kc * P:(kc + 1) * P],
                                qT[h2 * DH:(h2 + 1) * DH, hp, :],
                                start=True, stop=True)
                            et = pexp.tile([P, T], F32R, tag="e")
                            nc.scalar.activation(et[:], sc_ps[:], AF.Exp, scale=0.125)
                            nc.tensor.matmul(av_pair[h2][0:65], v_hp[:, kc, h2, :],
                                             et[:], start=(kc == 0), stop=(kc == 7))
                    for h2 in range(2):
                        av_ps = av_pair[h2]
                        den = pst.tile([1, T], F32, tag="st")
                        nc.scalar.activation(den[:], av_ps[64:65, :], AF.Identity)
                        recip = pst.tile([1, T], F32, tag="st")
                        nc.vector.reciprocal_approx_fast(recip[:], den[:])
                        rb = pbc.tile([64, T], F32, tag="rb2")
                        nc.gpsimd.partition_broadcast(rb[:], recip[:1, :])
                        nc.vector.tensor_tensor(
                            ctxT[h2 * DH:(h2 + 1) * DH, hp, :],
                            av_ps[0:DH, :], rb[:], OP.mult)

                # ---- Output projection + residuals ----
                for mcg in range(2):
                    ps = [psum_a.tile([P, T], F32, tag="pa", name=f"pa{_j}") for _j in range(4)]
                    for fc in range(FC):
                        wt = pw5.tile([P, 512], F32R, tag="w5")
                        nc.sync.dma_start(
                            wt[:], wo[i, fc * P:(fc + 1) * P, mcg * 512:(mcg + 1) * 512])
                        for j in range(4):
                            nc.tensor.matmul(ps[j][:], wt[:, j * P:(j + 1) * P],
                                             ctxT[:, fc, :],
                                             start=(fc == 0), stop=(fc == FC - 1))
                    for j in range(4):
                        mc = mcg * 4 + j
                        t1 = ptmp.tile([P, T], F32, tag="t1")
                        nc.vector.tensor_scalar_add(t1[:], ps[j][:], boc[:, mc:mc + 1])
                        nc.vector.tensor_tensor(x_sb[:, mc, :], x_sb[:, mc, :],
                                                t1[:], OP.add)
                for mc in range(FC):
                    nc.vector.tensor_tensor(x_sb[:, mc, :], x_sb[:, mc, :],
                                            ffacc[:, mc, :], OP.add)

            # ---- final LN -> output ----
            lfg = _load_col(nc, pcol, lnfg, FC, "c8")
            lfb = _load_col(nc, pcol, lnfb, FC, "c8")
            yln = pxn.tile([P, FC, T], F32R, tag="xn")
            _ln(nc, pools, x_sb, lfg, lfb, yln, ones)
            yT_v = yT.rearrange("(c p) t -> p c t", p=P)
            for fc in range(FC):
                nc.sync.dma_start(yT_v[:, fc, :], yln[:, fc, :].bitcast(F32))

    nc.compile()
    return nc


def _get_program():
    global _CACHED_NC
    if _CACHED_NC is None:
        _CACHED_NC = _build_program()
    return _CACHED_NC


def _numpy_fallback(x, mask, wq, bq, wk, bk, wv, bv, wo, bo, w1, b1, w2, b2,
                    ln1g, ln1b, ln2g, ln2b, lnfg, lnfb):
    def ln(t, g, b):
        m = t.mean(-1, keepdims=True)
        v = ((t - m) ** 2).mean(-1, keepdims=True)
        return (t - m) / np.sqrt(v + EPS) * g + b

    x = x.astype(np.float32).copy()
    Bn, Sn, Dm = x.shape
    scale = 1.0 / np.sqrt(np.float32(DH))
    maskb = (mask == 0)[:, None]
    for i in range(L):
        xn1 = ln(x, ln1g[i], ln1b[i])
        xn2 = ln(x, ln2g[i], ln2b[i])

        def heads(t):
            return t.reshape(Bn, Sn, H, DH).transpose(0, 2, 1, 3)

        q = heads(xn1 @ wq[i] + bq[i])
        k = heads(xn1 @ wk[i] + bk[i])
        v = heads(xn1 @ wv[i] + bv[i])
        sc = np.einsum("bhqd,bhkd->bhqk", q, k) * scale
        sc = np.where(maskb, np.float32(-1e9), sc)
        sc = sc - sc.max(-1, keepdims=True)
        e = np.exp(sc)
        attn = e / e.sum(-1, keepdims=True)
        ctx = np.einsum("bhqk,bhkd->bhqd", attn, v)
        ctx = ctx.transpose(0, 2, 1, 3).reshape(Bn, Sn, Dm)
        x = x + (ctx @ wo[i] + bo[i])
        h = np.maximum(xn2 @ w1[i] + b1[i], 0.0)
        x = x + (h @ w2[i] + b2[i])
    return ln(x, lnfg, lnfb)


def kernel(**inputs):
    x = np.asarray(inputs["x"], dtype=np.float32)
    mask = np.asarray(inputs["mask"])
    if not (mask == 1).all():
        return _numpy_fallback(**{k: np.asarray(v) for k, v in inputs.items()})

    nc = _get_program()

    shared = {}
    for name in ("wq", "wk", "wv", "wo", "w1", "w2", "bq", "bk", "bv", "bo",
                 "b1", "b2", "ln1g", "ln1b", "ln2g", "ln2b", "lnfg", "lnfb"):
        shared[name] = np.ascontiguousarray(np.asarray(inputs[name], dtype=np.float32))

    in_maps = []
    for c in range(NCORES):
        b, s = c // 2, c % 2
        xT = np.ascontiguousarray(x[b, s * T:(s + 1) * T, :].T)
        in_maps.append({"xT": xT, **shared})

    res = run_bass_kernel_spmd(nc, in_maps, list(range(NCORES)))

    out = np.empty((B, S, DM), dtype=np.float32)
    for c in range(NCORES):
        b, s = c // 2, c % 2
        out[b, s * T:(s + 1) * T, :] = res.results[c]["yT"].T
    return out


# revision 13
# speedup vs baseline: 1.0935x; 1.0375x over previous
"""Trainium2 Bass kernel for nn_Encoder (6-layer parallel-branch transformer encoder).

Sharding: sequence-split data-parallel over 8 cores. Core c owns the 512 tokens
[512*(c%2), 512*(c%2+1)) of batch element c//2. Per layer, core pairs (2b, 2b+1)
AllGather their K/V so each core attends over the full 1024-token sequence.
The attention mask is all-ones (verified at runtime; numpy fallback otherwise),
so key order is irrelevant and rank-indexed K/V blocks need no parity handling.

Layout: activations are kept feature-major ([features -> partitions, tokens ->
free]) end to end, which makes every projection a natural PE matmul with the
weight stationary and requires no on-device transposes. LayerNorm reductions
(over the partition axis) are done with ones-vector matmuls on the PE; softmax
denominators come for free from a ones-row appended to V in the AV matmul.

Matmuls run in float32r (full PE speed at N=512, ~2e-4 relative error/op).
"""

import sys

if "/opt/trn_rl_repo" not in sys.path:
    sys.path.insert(0, "/opt/trn_rl_repo")

import numpy as np

import concourse.bass as bass
import concourse.mybir as mybir
import concourse.tile as tile
from concourse import bacc
from concourse.bass_utils import run_bass_kernel_spmd

F32 = mybir.dt.float32
F32R = mybir.dt.float32r
AF = mybir.ActivationFunctionType
OP = mybir.AluOpType

P = 128          # partitions
B, S, DM, DFF = 4, 1024, 1024, 4096
H, DH = 16, 64   # heads, head dim
L = 6            # layers
T = 512          # tokens per core
FC = DM // P     # feature chunks (8)
HC = DFF // P    # ff chunks (32)
NCORES = 8
EPS = 1e-5
KV_K = DM * T          # floats in the kT region of the kv buffer
KV_V = T * DM          # floats in the v region
KV_TOT = KV_K + KV_V

_CACHED_NC = None


def _ln(nc, pools, xin, gcol, bcol, xn_out, ones):
    """LayerNorm over the feature (partition) axis, feature-major layout.

    xin: [128, FC, T] f32r; gcol/bcol: [128, FC] f32; xn_out: [128, FC, T].
    """
    psum_b, ptmp, pst, pbc = pools["psum_b"], pools["ptmp"], pools["pst"], pools["pbc"]

    sum_ps = psum_b.tile([P, T], F32, tag="pb")
    for fc in range(FC):
        nc.tensor.matmul(sum_ps[:1], ones[:], xin[:, fc, :],
                         start=(fc == 0), stop=(fc == FC - 1))
    sumsq_ps = psum_b.tile([P, T], F32, tag="pb")
    for fc in range(FC):
        sq = ptmp.tile([P, T], F32R, tag="sqr")
        nc.scalar.activation(sq[:], xin[:, fc, :], AF.Square)
        nc.tensor.matmul(sumsq_ps[:1], ones[:], sq[:],
                         start=(fc == 0), stop=(fc == FC - 1))

    m_sb = pst.tile([1, T], F32, tag="st")
    nc.vector.tensor_scalar_mul(m_sb[:], sum_ps[:1, :], 1.0 / DM)
    var = pst.tile([1, T], F32, tag="st")
    nc.vector.tensor_scalar_mul(var[:], sumsq_ps[:1, :], 1.0 / DM)
    mm = pst.tile([1, T], F32, tag="st")
    nc.vector.tensor_tensor(mm[:], m_sb[:], m_sb[:], OP.mult)
    nc.vector.tensor_tensor(var[:], var[:], mm[:], OP.subtract)
    nc.vector.tensor_scalar_add(var[:], var[:], EPS)
    rinv = pst.tile([1, T], F32, tag="st")
    nc.vector.reciprocal_approx_fast(rinv[:], var[:])
    r_sb = pst.tile([1, T], F32, tag="st")
    nc.scalar.activation(r_sb[:], rinv[:], AF.Sqrt)

    m_bt = pbc.tile([P, T], F32, tag="bc")
    r_bt = pbc.tile([P, T], F32, tag="bc")
    nc.gpsimd.partition_broadcast(m_bt[:], m_sb[:1, :])
    nc.gpsimd.partition_broadcast(r_bt[:], r_sb[:1, :])

    for fc in range(FC):
        t1 = ptmp.tile([P, T], F32, tag="t1")
        nc.vector.tensor_tensor(t1[:], xin[:, fc, :], m_bt[:], OP.subtract)
        nc.vector.tensor_tensor(t1[:], t1[:], r_bt[:], OP.mult)
        nc.scalar.activation(xn_out[:, fc, :], t1[:], AF.Identity,
                             bias=bcol[:, fc:fc + 1], scale=gcol[:, fc:fc + 1])


def _load_col(nc, pool, dram_vec, width, tag):
    """Load a [width*128] DRAM vector as a [128, width] per-partition column tile."""
    t = pool.tile([P, width], F32, tag=tag)
    nc.sync.dma_start(t[:], dram_vec.rearrange("(c p) -> p c", p=P))
    return t


def _build_program():
    nc = bacc.Bacc(None, target_bir_lowering=False, debug=False)

    xT = nc.dram_tensor("xT", [DM, T], F32R, kind="ExternalInput")
    wq = nc.dram_tensor("wq", [L, DM, DM], F32R, kind="ExternalInput")
    wk = nc.dram_tensor("wk", [L, DM, DM], F32R, kind="ExternalInput")
    wv = nc.dram_tensor("wv", [L, DM, DM], F32R, kind="ExternalInput")
    wo = nc.dram_tensor("wo", [L, DM, DM], F32R, kind="ExternalInput")
    w1 = nc.dram_tensor("w1", [L, DM, DFF], F32R, kind="ExternalInput")
    w2 = nc.dram_tensor("w2", [L, DFF, DM], F32R, kind="ExternalInput")
    bq = nc.dram_tensor("bq", [L, DM], F32, kind="ExternalInput")
    bk = nc.dram_tensor("bk", [L, DM], F32, kind="ExternalInput")
    bv = nc.dram_tensor("bv", [L, DM], F32, kind="ExternalInput")
    bo = nc.dram_tensor("bo", [L, DM], F32, kind="ExternalInput")
    b1 = nc.dram_tensor("b1", [L, DFF], F32, kind="ExternalInput")
    b2 = nc.dram_tensor("b2", [L, DM], F32, kind="ExternalInput")
    ln1g = nc.dram_tensor("ln1g", [L, DM], F32, kind="ExternalInput")
    ln1b = nc.dram_tensor("ln1b", [L, DM], F32, kind="ExternalInput")
    ln2g = nc.dram_tensor("ln2g", [L, DM], F32, kind="ExternalInput")
    ln2b = nc.dram_tensor("ln2b", [L, DM], F32, kind="ExternalInput")
    lnfg = nc.dram_tensor("lnfg", [DM], F32, kind="ExternalInput")
    lnfb = nc.dram_tensor("lnfb", [DM], F32, kind="ExternalInput")
    yT = nc.dram_tensor("yT", [DM, T], F32, kind="ExternalOutput")

    kv_send = [nc.dram_tensor(f"kv_send_{i}", [KV_TOT], F32) for i in range(L)]
    kv_recv = [nc.dram_tensor(f"kv_recv_{i}", [2, KV_TOT], F32) for i in range(L)]
    groups = [[0, 1], [2, 3], [4, 5], [6, 7]]

    from contextlib import ExitStack

    with tile.TileContext(nc) as tc:
        with ExitStack() as stack:
            ent = stack.enter_context
            px = ent(tc.tile_pool(name="px", bufs=1))
            pxn = ent(tc.tile_pool(name="pxn", bufs=2))
            pq = ent(tc.tile_pool(name="pq", bufs=1))
            pctx = ent(tc.tile_pool(name="pctx", bufs=1))
            pfacc = ent(tc.tile_pool(name="pfacc", bufs=1))
            pkv = ent(tc.tile_pool(name="pkv", bufs=3))
            pw5 = ent(tc.tile_pool(name="pw5", bufs=10))
            pw10 = ent(tc.tile_pool(name="pw10", bufs=5))
            pkhp = ent(tc.tile_pool(name="pkhp", bufs=2))
            pvhp = ent(tc.tile_pool(name="pvhp", bufs=2))
            pexp = ent(tc.tile_pool(name="pexp", bufs=4))
            ph = ent(tc.tile_pool(name="ph", bufs=5))
            pcol = ent(tc.tile_pool(name="pcol", bufs=10))
            pst = ent(tc.tile_pool(name="pst", bufs=5))
            pbv = ent(tc.tile_pool(name="pbv", bufs=1))
            pbc = ent(tc.tile_pool(name="pbc", bufs=2))
            ptmp = ent(tc.tile_pool(name="ptmp", bufs=2))
            pones = ent(tc.tile_pool(name="pones", bufs=1))
            psum_a = ent(tc.tile_pool(name="psum_a", bufs=4, space="PSUM"))
            psum_b = ent(tc.tile_pool(name="psum_b", bufs=3, space="PSUM"))

            pools = {"psum_b": psum_b, "ptmp": ptmp, "pst": pst, "pbc": pbc}

            ones_f = pones.tile([P, 1], F32)
            nc.vector.memset(ones_f[:], 1.0)
            ones = pones.tile([P, 1], F32R)
            nc.vector.tensor_copy(ones[:], ones_f[:])

            x_sb = px.tile([P, FC, T], F32R)
            nc.sync.dma_start(x_sb[:], xT.rearrange("(c p) t -> p c t", p=P))

            for i in range(L):
                # ---- per-layer constant columns ----
                l1g = _load_col(nc, pcol, ln1g[i], FC, "c8")
                l1b = _load_col(nc, pcol, ln1b[i], FC, "c8")
                l2g = _load_col(nc, pcol, ln2g[i], FC, "c8")
                l2b = _load_col(nc, pcol, ln2b[i], FC, "c8")
                bqc = _load_col(nc, pcol, bq[i], FC, "c8")
                bkc = _load_col(nc, pcol, bk[i], FC, "c8")
                boc = _load_col(nc, pcol, bo[i], FC, "c8")
                b2c = _load_col(nc, pcol, b2[i], FC, "c8")
                b1c = _load_col(nc, pcol, b1[i], HC, "c32")

                # ---- LN1 (attention branch input) ----
                xn1 = pxn.tile([P, FC, T], F32R, tag="xn")
                _ln(nc, pools, x_sb, l1g, l1b, xn1, ones)
                xn2 = pxn.tile([P, FC, T], F32R, tag="xn")
                _ln(nc, pools, x_sb, l2g, l2b, xn2, ones)

                # ---- K projection (own tokens) -> kv_send ----
                send_k = kv_send[i][0:KV_K].rearrange("(d t) -> d t", t=T)
                for mcg in range(2):
                    ps = [psum_a.tile([P, T], F32, tag="pa", name=f"pa{_j}") for _j in range(4)]
                    for fc in range(FC):
                        wt = pw5.tile([P, 512], F32R, tag="w5")
                        nc.sync.dma_start(
                            wt[:], wk[i, fc * P:(fc + 1) * P, mcg * 512:(mcg + 1) * 512])
                        for j in range(4):
                            nc.tensor.matmul(ps[j][:], wt[:, j * P:(j + 1) * P],
                                             xn1[:, fc, :],
                                             start=(fc == 0), stop=(fc == FC - 1))
                    for j in range(4):
                        mc = mcg * 4 + j
                        kvt = pkv.tile([P, T], F32, tag="kv")
                        nc.vector.tensor_scalar_add(kvt[:], ps[j][:], bkc[:, mc:mc + 1])
                        nc.sync.dma_start(send_k[mc * P:(mc + 1) * P, :], kvt[:])

                # ---- V projection (own tokens, token-major out) -> kv_send ----
                send_v = kv_send[i][KV_K:].rearrange("(tk d) -> tk d", d=DM)
                for dh2 in range(2):
                    bvrow = pbv.tile([1, 512], F32, tag="bvr")
                    nc.sync.dma_start(bvrow[:], bv[i, dh2 * 512:(dh2 + 1) * 512][None, :])
                    bv_bt = pbc.tile([P, 512], F32, tag="bc")
                    nc.gpsimd.partition_broadcast(bv_bt[:], bvrow[:1, :])
                    ps = [psum_a.tile([P, 512], F32, tag="pa", name=f"pa{_j}") for _j in range(4)]
                    for fc in range(FC):
                        wt = pw5.tile([P, 512], F32R, tag="w5")
                        nc.sync.dma_start(
                            wt[:], wv[i, fc * P:(fc + 1) * P, dh2 * 512:(dh2 + 1) * 512])
                        for tc4 in range(4):
                            nc.tensor.matmul(ps[tc4][:],
                                             xn1[:, fc, tc4 * P:(tc4 + 1) * P], wt[:],
                                             start=(fc == 0), stop=(fc == FC - 1))
                    for tc4 in range(4):
                        kvt = pkv.tile([P, 512], F32, tag="kv")
                        nc.vector.tensor_tensor(kvt[:], ps[tc4][:], bv_bt[:], OP.add)
                        nc.sync.dma_start(
                            send_v[tc4 * P:(tc4 + 1) * P, dh2 * 512:(dh2 + 1) * 512],
                            kvt[:])

                # ---- AllGather K/V within the core pair ----
                nc.gpsimd.collective_compute(
                    "AllGather", OP.bypass,
                    ins=[kv_send[i][:]], outs=[kv_recv[i][:]],
                    replica_groups=groups)

                # ---- Q projection (own tokens), feature-major out ----
                qT = pq.tile([P, FC, T], F32R)
                for mcg in range(2):
                    ps = [psum_a.tile([P, T], F32, tag="pa", name=f"pa{_j}") for _j in range(4)]
                    for fc in range(FC):
                        wt = pw5.tile([P, 512], F32R, tag="w5")
                        nc.sync.dma_start(
                            wt[:], wq[i, fc * P:(fc + 1) * P, mcg * 512:(mcg + 1) * 512])
                        for j in range(4):
                            nc.tensor.matmul(ps[j][:], wt[:, j * P:(j + 1) * P],
                                             xn1[:, fc, :],
                                             start=(fc == 0), stop=(fc == FC - 1))
                    for j in range(4):
                        mc = mcg * 4 + j
                        nc.vector.tensor_scalar_add(qT[:, mc, :], ps[j][:],
                                                    bqc[:, mc:mc + 1])

                # ---- FFN branch (xn2 computed up top) ----
                ffacc = pfacc.tile([P, FC, T], F32)
                for g in range(8):
                    # h chunks hc = 4g..4g+3
                    ps = [psum_a.tile([P, T], F32, tag="pa", name=f"pa{_j}") for _j in range(4)]
                    for fc in range(FC):
                        wt = pw5.tile([P, 512], F32R, tag="w5")
                        nc.sync.dma_start(
                            wt[:], w1[i, fc * P:(fc + 1) * P, g * 512:(g + 1) * 512])
                        for j in range(4):
                            nc.tensor.matmul(ps[j][:], wt[:, j * P:(j + 1) * P],
                                             xn2[:, fc, :],
                                             start=(fc == 0), stop=(fc == FC - 1))
                    hts = []
                    for j in range(4):
                        hc = g * 4 + j
                        ht = ph.tile([P, T], F32R, tag="h")
                        nc.scalar.activation(ht[:], ps[j][:], AF.Relu,
                                             bias=b1c[:, hc:hc + 1])
                        hts.append(ht)
                    w2ts = []
                    for j in range(4):
                        hc = g * 4 + j
                        w2t = pw10.tile([P, DM], F32R, tag="w10")
                        nc.sync.dma_start(w2t[:], w2[i, hc * P:(hc + 1) * P, :])
                        w2ts.append(w2t)
                    for mc in range(FC):
                        wps = psum_b.tile([P, T], F32, tag="pb")
                        for j in range(4):
                            nc.tensor.matmul(wps[:], w2ts[j][:, mc * P:(mc + 1) * P],
                                             hts[j][:],
                                             start=(j == 0), stop=(j == 3))
                        if g == 0:
                            nc.vector.tensor_scalar_add(ffacc[:, mc, :], wps[:],
                                                        b2c[:, mc:mc + 1])
                        else:
                            nc.vector.tensor_tensor(ffacc[:, mc, :], ffacc[:, mc, :],
                                                    wps[:], OP.add)

                # ---- Attention over gathered K/V ----
                recv_k = [kv_recv[i][r, 0:KV_K].rearrange("(d t) -> d t", t=T)
                          for r in range(2)]
                recv_v = [kv_recv[i][r, KV_K:].rearrange("(tk d) -> tk d", d=DM)
                          for r in range(2)]
                ctxT = pctx.tile([P, FC, T], F32R)
                for hp in range(FC):
                    kT_hp = pkhp.tile([P, 2 * T], F32R, tag="khp")
                    for r in range(2):
                        nc.sync.dma_start(
                            kT_hp[:, r * T:(r + 1) * T],
                            recv_k[r][hp * P:(hp + 1) * P, :].bitcast(F32R))
                    v_hp = pvhp.tile([P, 8, 2, 65], F32R, tag="vhp")
                    for r in range(2):
                        for h2 in range(2):
                            src = recv_v[r][:, hp * P + h2 * DH: hp * P + (h2 + 1) * DH]
                            src = src.rearrange("(tc p) d -> p tc d", p=P)
                            nc.sync.dma_start(v_hp[:, r * 4:(r + 1) * 4, h2, 0:64],
                                              src.bitcast(F32R))
                    nc.vector.tensor_copy(
                        v_hp[:, :, :, 64:65],
                        ones[:, :1][:, None, None, :].to_broadcast([P, 8, 2, 1]))

                    av_pair = [psum_b.tile([P, T], F32, tag="pb", name=f"av{_h}")
                               for _h in range(2)]
                    ets = {}
                    for kc in range(9):
                        for h2 in range(2):
                            if kc < 8:
                                sc_ps = psum_a.tile([P, T], F32, tag="pa")
                                nc.tensor.matmul(
                                    sc_ps[:],
                                    kT_hp[h2 * DH:(h2 + 1) * DH, kc * P:(kc + 1) * P],
                                    qT[h2 * DH:(h2 + 1) * DH, hp, :],
                                    start=True, stop=True)
                                et = pexp.tile([P, T], F32R, tag="e")
                                nc.scalar.activation(et[:], sc_ps[:], AF.Exp,
                                                     scale=0.125)
                                ets[(kc, h2)] = et
                            if kc > 0:
                                nc.tensor.matmul(av_pair[h2][0:65],
                                                 v_hp[:, kc - 1, h2, :],
                                                 ets.pop((kc - 1, h2)),
                                                 start=(kc == 1), stop=(kc == 8))
                    for h2 in range(2):
                        av_ps = av_pair[h2]
                        den = pst.tile([1, T], F32, tag="st")
                        nc.scalar.activation(den[:], av_ps[64:65, :], AF.Identity)
                        recip = pst.tile([1, T], F32, tag="st")
                        nc.vector.reciprocal_approx_fast(recip[:], den[:])
                        rb = pbc.tile([64, T], F32, tag="rb2")
                        nc.gpsimd.partition_broadcast(rb[:], recip[:1, :])
                        nc.vector.tensor_tensor(
                            ctxT[h2 * DH:(h2 + 1) * DH, hp, :],
                            av_ps[0:DH, :], rb[:], OP.mult)

                # ---- Output projection + residuals ----
                for mcg in range(2):
                    ps = [psum_a.tile([P, T], F32, tag="pa", name=f"pa{_j}") for _j in range(4)]
                    for fc in range(FC):
                        wt = pw5.tile([P, 512], F32R, tag="w5")
                        nc.sync.dma_start(
                            wt[:], wo[i, fc * P:(fc + 1) * P, mcg * 512:(mcg + 1) * 512])
                        for j in range(4):
                            nc.tensor.matmul(ps[j][:], wt[:, j * P:(j + 1) * P],
                                             ctxT[:, fc, :],
                                             start=(fc == 0), stop=(fc == FC - 1))
                    for j in range(4):
                        mc = mcg * 4 + j
                        t1 = ptmp.tile([P, T], F32, tag="t1")
                        nc.vector.tensor_scalar_add(t1[:], ps[j][:], boc[:, mc:mc + 1])
                        nc.vector.tensor_tensor(x_sb[:, mc, :], x_sb[:, mc, :],
                                                t1[:], OP.add)
                for mc in range(FC):
                    nc.vector.tensor_tensor(x_sb[:, mc, :], x_sb[:, mc, :],
                                            ffacc[:, mc, :], OP.add)

            # ---- final LN -> output ----
            lfg = _load_col(nc, pcol, lnfg, FC, "c8")
            lfb = _load_col(nc, pcol, lnfb, FC, "c8")
            yln = pxn.tile([P, FC, T], F32R, tag="xn")
            _ln(nc, pools, x_sb, lfg, lfb, yln, ones)
            yT_v = yT.rearrange("(c p) t -> p c t", p=P)
            for fc in range(FC):
                nc.sync.dma_start(yT_v[:, fc, :], yln[:, fc, :].bitcast(F32))

    nc.compile()
    return nc


def _get_program():
    global _CACHED_NC
    if _CACHED_NC is None:
        _CACHED_NC = _build_program()
    return _CACHED_NC


def _numpy_fallback(x, mask, wq, bq, wk, bk, wv, bv, wo, bo, w1, b1, w2, b2,
                    ln1g, ln1b, ln2g, ln2b, lnfg, lnfb):
    def ln(t, g, b):
        m = t.mean(-1, keepdims=True)
        v = ((t - m) ** 2).mean(-1, keepdims=True)
        return (t - m) / np.sqrt(v + EPS) * g + b

    x = x.astype(np.float32).copy()
    Bn, Sn, Dm = x.shape
    scale = 1.0 / np.sqrt(np.float32(DH))
    maskb = (mask == 0)[:, None]
    for i in range(L):
        xn1 = ln(x, ln1g[i], ln1b[i])
        xn2 = ln(x, ln2g[i], ln2b[i])

        def heads(t):
            return t.reshape(Bn, Sn, H, DH).transpose(0, 2, 1, 3)

        q = heads(xn1 @ wq[i] + bq[i])
        k = heads(xn1 @ wk[i] + bk[i])
        v = heads(xn1 @ wv[i] + bv[i])
        sc = np.einsum("bhqd,bhkd->bhqk", q, k) * scale
        sc = np.where(maskb, np.float32(-1e9), sc)
        sc = sc - sc.max(-1, keepdims=True)
        e = np.exp(sc)
        attn = e / e.sum(-1, keepdims=True)
        ctx = np.einsum("bhqk,bhkd->bhqd", attn, v)
        ctx = ctx.transpose(0, 2, 1, 3).reshape(Bn, Sn, Dm)
        x = x + (ctx @ wo[i] + bo[i])
        h = np.maximum(xn2 @ w1[i] + b1[i], 0.0)
        x = x + (h @ w2[i] + b2[i])
    return ln(x, lnfg, lnfb)


def kernel(**inputs):
    x = np.asarray(inputs["x"], dtype=np.float32)
    mask = np.asarray(inputs["mask"])
    if not (mask == 1).all():
        return _numpy_fallback(**{k: np.asarray(v) for k, v in inputs.items()})

    nc = _get_program()

    shared = {}
    for name in ("wq", "wk", "wv", "wo", "w1", "w2", "bq", "bk", "bv", "bo",
                 "b1", "b2", "ln1g", "ln1b", "ln2g", "ln2b", "lnfg", "lnfb"):
        shared[name] = np.ascontiguousarray(np.asarray(inputs[name], dtype=np.float32))

    in_maps = []
    for c in range(NCORES):
        b, s = c // 2, c % 2
        xT = np.ascontiguousarray(x[b, s * T:(s + 1) * T, :].T)
        in_maps.append({"xT": xT, **shared})

    res = run_bass_kernel_spmd(nc, in_maps, list(range(NCORES)))

    out = np.empty((B, S, DM), dtype=np.float32)
    for c in range(NCORES):
        b, s = c // 2, c % 2
        out[b, s * T:(s + 1) * T, :] = res.results[c]["yT"].T
    return out


# revision 15
# speedup vs baseline: 1.4372x; 1.3143x over previous
"""Trainium2 Bass kernel for nn_Encoder (6-layer parallel-branch transformer encoder).

Sharding: sequence-split data-parallel over 8 cores. Core c owns the 512 tokens
[512*(c%2), 512*(c%2+1)) of batch element c//2. Per layer, core pairs (2b, 2b+1)
AllGather their K/V so each core attends over the full 1024-token sequence.
The attention mask is all-ones (verified at runtime; numpy fallback otherwise),
so key order is irrelevant and rank-indexed K/V blocks need no parity handling.

Layout: activations are kept feature-major ([features -> partitions, tokens ->
free]) end to end, which makes every projection a natural PE matmul with the
weight stationary and requires no on-device transposes. LayerNorm reductions
(over the partition axis) are done with ones-vector matmuls on the PE; softmax
denominators come for free from a ones-row appended to V in the AV matmul.

Matmuls run in float32r (full PE speed at N=512, ~2e-4 relative error/op).
"""

import sys

if "/opt/trn_rl_repo" not in sys.path:
    sys.path.insert(0, "/opt/trn_rl_repo")

import numpy as np

import concourse.bass as bass
import concourse.mybir as mybir
import concourse.tile as tile
from concourse import bacc
from concourse.bass_utils import run_bass_kernel_spmd

F32 = mybir.dt.float32
F32R = mybir.dt.float32r
F16 = mybir.dt.float16
AF = mybir.ActivationFunctionType
OP = mybir.AluOpType

P = 128          # partitions
B, S, DM, DFF = 4, 1024, 1024, 4096
H, DH = 16, 64   # heads, head dim
L = 6            # layers
T = 512          # tokens per core
FC = DM // P     # feature chunks (8)
HC = DFF // P    # ff chunks (32)
NCORES = 8
EPS = 1e-5
KV_K = DM * T          # floats in the kT region of the kv buffer
KV_V = T * DM          # floats in the v region
KV_TOT = KV_K + KV_V

_CACHED_NC = None


def _ln(nc, pools, xin, gcol, bcol, xn_out, ones):
    """LayerNorm over the feature (partition) axis, feature-major layout.

    xin: [128, FC, T] f32r; gcol/bcol: [128, FC] f32; xn_out: [128, FC, T].
    """
    psum_b, ptmp, pst, pbc = pools["psum_b"], pools["ptmp"], pools["pst"], pools["pbc"]

    sum_ps = psum_b.tile([P, T], F32, tag="pb")
    for fc in range(FC):
        nc.tensor.matmul(sum_ps[:1], ones[:], xin[:, fc, :],
                         start=(fc == 0), stop=(fc == FC - 1))
    sumsq_ps = psum_b.tile([P, T], F32, tag="pb")
    for fc in range(FC):
        sq = ptmp.tile([P, T], F32R, tag="sqr")
        nc.scalar.activation(sq[:], xin[:, fc, :], AF.Square)
        nc.tensor.matmul(sumsq_ps[:1], ones[:], sq[:],
                         start=(fc == 0), stop=(fc == FC - 1))

    m_sb = pst.tile([1, T], F32, tag="st")
    nc.vector.tensor_scalar_mul(m_sb[:], sum_ps[:1, :], 1.0 / DM)
    var = pst.tile([1, T], F32, tag="st")
    nc.vector.tensor_scalar_mul(var[:], sumsq_ps[:1, :], 1.0 / DM)
    mm = pst.tile([1, T], F32, tag="st")
    nc.vector.tensor_tensor(mm[:], m_sb[:], m_sb[:], OP.mult)
    nc.vector.tensor_tensor(var[:], var[:], mm[:], OP.subtract)
    nc.vector.tensor_scalar_add(var[:], var[:], EPS)
    rinv = pst.tile([1, T], F32, tag="st")
    nc.vector.reciprocal_approx_fast(rinv[:], var[:])
    r_sb = pst.tile([1, T], F32, tag="st")
    nc.scalar.activation(r_sb[:], rinv[:], AF.Sqrt)

    m_bt = pbc.tile([P, T], F32, tag="bc")
    r_bt = pbc.tile([P, T], F32, tag="bc")
    nc.gpsimd.partition_broadcast(m_bt[:], m_sb[:1, :])
    nc.gpsimd.partition_broadcast(r_bt[:], r_sb[:1, :])

    for fc in range(FC):
        t1 = ptmp.tile([P, T], F32, tag="t1")
        nc.vector.tensor_tensor(t1[:], xin[:, fc, :], m_bt[:], OP.subtract)
        nc.vector.tensor_tensor(t1[:], t1[:], r_bt[:], OP.mult)
        nc.scalar.activation(xn_out[:, fc, :], t1[:], AF.Identity,
                             bias=bcol[:, fc:fc + 1], scale=gcol[:, fc:fc + 1])


def _load_col(nc, pool, dram_vec, width, tag):
    """Load a [width*128] DRAM vector as a [128, width] per-partition column tile."""
    t = pool.tile([P, width], F32, tag=tag)
    nc.sync.dma_start(t[:], dram_vec.rearrange("(c p) -> p c", p=P))
    return t


def _build_program():
    nc = bacc.Bacc(None, target_bir_lowering=False, debug=False)

    xT = nc.dram_tensor("xT", [DM, T], F32R, kind="ExternalInput")
    wq = nc.dram_tensor("wq", [L, DM, DM], F16, kind="ExternalInput")
    wk = nc.dram_tensor("wk", [L, DM, DM], F16, kind="ExternalInput")
    wv = nc.dram_tensor("wv", [L, DM, DM], F16, kind="ExternalInput")
    wo = nc.dram_tensor("wo", [L, DM, DM], F16, kind="ExternalInput")
    w1 = nc.dram_tensor("w1", [L, DM, DFF], F16, kind="ExternalInput")
    w2 = nc.dram_tensor("w2", [L, DFF, DM], F16, kind="ExternalInput")
    bq = nc.dram_tensor("bq", [L, DM], F32, kind="ExternalInput")
    bk = nc.dram_tensor("bk", [L, DM], F32, kind="ExternalInput")
    bv = nc.dram_tensor("bv", [L, DM], F32, kind="ExternalInput")
    bo = nc.dram_tensor("bo", [L, DM], F32, kind="ExternalInput")
    b1 = nc.dram_tensor("b1", [L, DFF], F32, kind="ExternalInput")
    b2 = nc.dram_tensor("b2", [L, DM], F32, kind="ExternalInput")
    ln1g = nc.dram_tensor("ln1g", [L, DM], F32, kind="ExternalInput")
    ln1b = nc.dram_tensor("ln1b", [L, DM], F32, kind="ExternalInput")
    ln2g = nc.dram_tensor("ln2g", [L, DM], F32, kind="ExternalInput")
    ln2b = nc.dram_tensor("ln2b", [L, DM], F32, kind="ExternalInput")
    lnfg = nc.dram_tensor("lnfg", [DM], F32, kind="ExternalInput")
    lnfb = nc.dram_tensor("lnfb", [DM], F32, kind="ExternalInput")
    yT = nc.dram_tensor("yT", [DM, T], F32, kind="ExternalOutput")

    kv_send = [nc.dram_tensor(f"kv_send_{i}", [KV_TOT], F16) for i in range(L)]
    kv_recv = [nc.dram_tensor(f"kv_recv_{i}", [2, KV_TOT], F16) for i in range(L)]
    groups = [[0, 1], [2, 3], [4, 5], [6, 7]]

    from contextlib import ExitStack

    with tile.TileContext(nc) as tc:
        with ExitStack() as stack:
            ent = stack.enter_context
            px = ent(tc.tile_pool(name="px", bufs=1))
            pxn = ent(tc.tile_pool(name="pxn", bufs=2))
            pq = ent(tc.tile_pool(name="pq", bufs=1))
            pctx = ent(tc.tile_pool(name="pctx", bufs=1))
            pfacc = ent(tc.tile_pool(name="pfacc", bufs=1))
            pkv = ent(tc.tile_pool(name="pkv", bufs=3))
            pw5 = ent(tc.tile_pool(name="pw5", bufs=10))
            pw10 = ent(tc.tile_pool(name="pw10", bufs=5))
            pkhp = ent(tc.tile_pool(name="pkhp", bufs=2))
            pvhp = ent(tc.tile_pool(name="pvhp", bufs=2))
            pexp = ent(tc.tile_pool(name="pexp", bufs=4))
            ph = ent(tc.tile_pool(name="ph", bufs=5))
            pcol = ent(tc.tile_pool(name="pcol", bufs=10))
            pst = ent(tc.tile_pool(name="pst", bufs=5))
            pbv = ent(tc.tile_pool(name="pbv", bufs=1))
            pbc = ent(tc.tile_pool(name="pbc", bufs=2))
            ptmp = ent(tc.tile_pool(name="ptmp", bufs=2))
            pones = ent(tc.tile_pool(name="pones", bufs=1))
            psum_a = ent(tc.tile_pool(name="psum_a", bufs=4, space="PSUM"))
            psum_b = ent(tc.tile_pool(name="psum_b", bufs=3, space="PSUM"))

            pools = {"psum_b": psum_b, "ptmp": ptmp, "pst": pst, "pbc": pbc}

            ones_f = pones.tile([P, 1], F32)
            nc.vector.memset(ones_f[:], 1.0)
            ones = pones.tile([P, 1], F32R)
            nc.vector.tensor_copy(ones[:], ones_f[:])
            ones16 = pones.tile([P, 1], F16)
            nc.vector.tensor_copy(ones16[:], ones_f[:])

            x_sb = px.tile([P, FC, T], F32R)
            nc.sync.dma_start(x_sb[:], xT.rearrange("(c p) t -> p c t", p=P))

            for i in range(L):
                # ---- per-layer constant columns ----
                l1g = _load_col(nc, pcol, ln1g[i], FC, "c8")
                l1b = _load_col(nc, pcol, ln1b[i], FC, "c8")
                l2g = _load_col(nc, pcol, ln2g[i], FC, "c8")
                l2b = _load_col(nc, pcol, ln2b[i], FC, "c8")
                bqc = _load_col(nc, pcol, bq[i], FC, "c8")
                bkc = _load_col(nc, pcol, bk[i], FC, "c8")
                boc = _load_col(nc, pcol, bo[i], FC, "c8")
                b2c = _load_col(nc, pcol, b2[i], FC, "c8")
                b1c = _load_col(nc, pcol, b1[i], HC, "c32")

                # ---- LN1 (attention branch input) ----
                xn1 = pxn.tile([P, FC, T], F16, tag="xn")
                _ln(nc, pools, x_sb, l1g, l1b, xn1, ones)
                xn2 = pxn.tile([P, FC, T], F16, tag="xn")
                _ln(nc, pools, x_sb, l2g, l2b, xn2, ones)

                # ---- K projection (own tokens) -> kv_send ----
                send_k = kv_send[i][0:KV_K].rearrange("(d t) -> d t", t=T)
                for mcg in range(2):
                    ps = [psum_a.tile([P, T], F32, tag="pa", name=f"pa{_j}") for _j in range(4)]
                    for fc in range(FC):
                        wt = pw5.tile([P, 512], F16, tag="w5")
                        nc.sync.dma_start(
                            wt[:], wk[i, fc * P:(fc + 1) * P, mcg * 512:(mcg + 1) * 512])
                        for j in range(4):
                            nc.tensor.matmul(ps[j][:], wt[:, j * P:(j + 1) * P],
                                             xn1[:, fc, :],
                                             start=(fc == 0), stop=(fc == FC - 1))
                    for j in range(4):
                        mc = mcg * 4 + j
                        kvt = pkv.tile([P, T], F16, tag="kv")
                        nc.vector.tensor_scalar_add(kvt[:], ps[j][:], bkc[:, mc:mc + 1])
                        nc.sync.dma_start(send_k[mc * P:(mc + 1) * P, :], kvt[:])

                # ---- V projection (own tokens, token-major out) -> kv_send ----
                send_v = kv_send[i][KV_K:].rearrange("(tk d) -> tk d", d=DM)
                for dh2 in range(2):
                    bvrow = pbv.tile([1, 512], F32, tag="bvr")
                    nc.sync.dma_start(bvrow[:], bv[i, dh2 * 512:(dh2 + 1) * 512][None, :])
                    bv_bt = pbc.tile([P, 512], F32, tag="bc")
                    nc.gpsimd.partition_broadcast(bv_bt[:], bvrow[:1, :])
                    ps = [psum_a.tile([P, 512], F32, tag="pa", name=f"pa{_j}") for _j in range(4)]
                    for fc in range(FC):
                        wt = pw5.tile([P, 512], F16, tag="w5")
                        nc.sync.dma_start(
                            wt[:], wv[i, fc * P:(fc + 1) * P, dh2 * 512:(dh2 + 1) * 512])
                        for tc4 in range(4):
                            nc.tensor.matmul(ps[tc4][:],
                                             xn1[:, fc, tc4 * P:(tc4 + 1) * P], wt[:],
                                             start=(fc == 0), stop=(fc == FC - 1))
                    for tc4 in range(4):
                        kvt = pkv.tile([P, 512], F16, tag="kv")
                        nc.vector.tensor_tensor(kvt[:], ps[tc4][:], bv_bt[:], OP.add)
                        nc.sync.dma_start(
                            send_v[tc4 * P:(tc4 + 1) * P, dh2 * 512:(dh2 + 1) * 512],
                            kvt[:])

                # ---- AllGather K/V within the core pair ----
                nc.gpsimd.collective_compute(
                    "AllGather", OP.bypass,
                    ins=[kv_send[i][:]], outs=[kv_recv[i][:]],
                    replica_groups=groups)

                # ---- Q projection (own tokens), feature-major out ----
                qT = pq.tile([P, FC, T], F16)
                for mcg in range(2):
                    ps = [psum_a.tile([P, T], F32, tag="pa", name=f"pa{_j}") for _j in range(4)]
                    for fc in range(FC):
                        wt = pw5.tile([P, 512], F16, tag="w5")
                        nc.sync.dma_start(
                            wt[:], wq[i, fc * P:(fc + 1) * P, mcg * 512:(mcg + 1) * 512])
                        for j in range(4):
                            nc.tensor.matmul(ps[j][:], wt[:, j * P:(j + 1) * P],
                                             xn1[:, fc, :],
                                             start=(fc == 0), stop=(fc == FC - 1))
                    for j in range(4):
                        mc = mcg * 4 + j
                        nc.vector.tensor_scalar_add(qT[:, mc, :], ps[j][:],
                                                    bqc[:, mc:mc + 1])

                # ---- FFN branch (xn2 computed up top) ----
                ffacc = pfacc.tile([P, FC, T], F32)
                for g in range(8):
                    # h chunks hc = 4g..4g+3
                    ps = [psum_a.tile([P, T], F32, tag="pa", name=f"pa{_j}") for _j in range(4)]
                    for fc in range(FC):
                        wt = pw5.tile([P, 512], F16, tag="w5")
                        nc.sync.dma_start(
                            wt[:], w1[i, fc * P:(fc + 1) * P, g * 512:(g + 1) * 512])
                        for j in range(4):
                            nc.tensor.matmul(ps[j][:], wt[:, j * P:(j + 1) * P],
                                             xn2[:, fc, :],
                                             start=(fc == 0), stop=(fc == FC - 1))
                    hts = []
                    for j in range(4):
                        hc = g * 4 + j
                        ht = ph.tile([P, T], F16, tag="h")
                        nc.scalar.activation(ht[:], ps[j][:], AF.Relu,
                                             bias=b1c[:, hc:hc + 1])
                        hts.append(ht)
                    w2ts = []
                    for j in range(4):
                        hc = g * 4 + j
                        w2t = pw10.tile([P, DM], F16, tag="w10")
                        nc.sync.dma_start(w2t[:], w2[i, hc * P:(hc + 1) * P, :])
                        w2ts.append(w2t)
                    for mc in range(FC):
                        wps = psum_b.tile([P, T], F32, tag="pb")
                        for j in range(4):
                            nc.tensor.matmul(wps[:], w2ts[j][:, mc * P:(mc + 1) * P],
                                             hts[j][:],
                                             start=(j == 0), stop=(j == 3))
                        if g == 0:
                            nc.vector.tensor_scalar_add(ffacc[:, mc, :], wps[:],
                                                        b2c[:, mc:mc + 1])
                        else:
                            nc.vector.tensor_tensor(ffacc[:, mc, :], ffacc[:, mc, :],
                                                    wps[:], OP.add)

                # ---- Attention over gathered K/V ----
                recv_k = [kv_recv[i][r, 0:KV_K].rearrange("(d t) -> d t", t=T)
                          for r in range(2)]
                recv_v = [kv_recv[i][r, KV_K:].rearrange("(tk d) -> tk d", d=DM)
                          for r in range(2)]
                ctxT = pctx.tile([P, FC, T], F16)
                for hp in range(FC):
                    kT_hp = pkhp.tile([P, 2 * T], F16, tag="khp")
                    for r in range(2):
                        nc.sync.dma_start(
                            kT_hp[:, r * T:(r + 1) * T],
                            recv_k[r][hp * P:(hp + 1) * P, :])
                    v_hp = pvhp.tile([P, 8, 2, 65], F16, tag="vhp")
                    for r in range(2):
                        for h2 in range(2):
                            src = recv_v[r][:, hp * P + h2 * DH: hp * P + (h2 + 1) * DH]
                            src = src.rearrange("(tc p) d -> p tc d", p=P)
                            nc.sync.dma_start(v_hp[:, r * 4:(r + 1) * 4, h2, 0:64],
                                              src)
                    nc.vector.tensor_copy(
                        v_hp[:, :, :, 64:65],
                        ones16[:, :1][:, None, None, :].to_broadcast([P, 8, 2, 1]))

                    av_pair = [psum_b.tile([P, T], F32, tag="pb", name=f"av{_h}")
                               for _h in range(2)]
                    ets = {}
                    for kc in range(9):
                        for h2 in range(2):
                            if kc < 8:
                                sc_ps = psum_a.tile([P, T], F32, tag="pa")
                                nc.tensor.matmul(
                                    sc_ps[:],
                                    kT_hp[h2 * DH:(h2 + 1) * DH, kc * P:(kc + 1) * P],
                                    qT[h2 * DH:(h2 + 1) * DH, hp, :],
                                    start=True, stop=True)
                                et = pexp.tile([P, T], F16, tag="e")
                                nc.scalar.activation(et[:], sc_ps[:], AF.Exp,
                                                     scale=0.125)
                                ets[(kc, h2)] = et
                            if kc > 0:
                                nc.tensor.matmul(av_pair[h2][0:65],
                                                 v_hp[:, kc - 1, h2, :],
                                                 ets.pop((kc - 1, h2)),
                                                 start=(kc == 1), stop=(kc == 8))
                    for h2 in range(2):
                        av_ps = av_pair[h2]
                        den = pst.tile([1, T], F32, tag="st")
                        nc.scalar.activation(den[:], av_ps[64:65, :], AF.Identity)
                        recip = pst.tile([1, T], F32, tag="st")
                        nc.vector.reciprocal_approx_fast(recip[:], den[:])
                        rb = pbc.tile([64, T], F32, tag="rb2")
                        nc.gpsimd.partition_broadcast(rb[:], recip[:1, :])
                        nc.vector.tensor_tensor(
                            ctxT[h2 * DH:(h2 + 1) * DH, hp, :],
                            av_ps[0:DH, :], rb[:], OP.mult)

                # ---- Output projection + residuals ----
                for mcg in range(2):
                    ps = [psum_a.tile([P, T], F32, tag="pa", name=f"pa{_j}") for _j in range(4)]
                    for fc in range(FC):
                        wt = pw5.tile([P, 512], F16, tag="w5")
                        nc.sync.dma_start(
                            wt[:], wo[i, fc * P:(fc + 1) * P, mcg * 512:(mcg + 1) * 512])
                        for j in range(4):
                            nc.tensor.matmul(ps[j][:], wt[:, j * P:(j + 1) * P],
                                             ctxT[:, fc, :],
                                             start=(fc == 0), stop=(fc == FC - 1))
                    for j in range(4):
                        mc = mcg * 4 + j
                        t1 = ptmp.tile([P, T], F32, tag="t1")
                        nc.vector.tensor_scalar_add(t1[:], ps[j][:], boc[:, mc:mc + 1])
                        nc.vector.tensor_tensor(x_sb[:, mc, :], x_sb[:, mc, :],
                                                t1[:], OP.add)
                for mc in range(FC):
                    nc.vector.tensor_tensor(x_sb[:, mc, :], x_sb[:, mc, :],
                                            ffacc[:, mc, :], OP.add)

            # ---- final LN -> output ----
            lfg = _load_col(nc, pcol, lnfg, FC, "c8")
            lfb = _load_col(nc, pcol, lnfb, FC, "c8")
            yln = px.tile([P, FC, T], F32, tag="yln")
            _ln(nc, pools, x_sb, lfg, lfb, yln, ones)
            yT_v = yT.rearrange("(c p) t -> p c t", p=P)
            for fc in range(FC):
                nc.sync.dma_start(yT_v[:, fc, :], yln[:, fc, :])

    nc.compile()
    return nc


def _get_program():
    global _CACHED_NC
    if _CACHED_NC is None:
        _CACHED_NC = _build_program()
    return _CACHED_NC


def _numpy_fallback(x, mask, wq, bq, wk, bk, wv, bv, wo, bo, w1, b1, w2, b2,
                    ln1g, ln1b, ln2g, ln2b, lnfg, lnfb):
    def ln(t, g, b):
        m = t.mean(-1, keepdims=True)
        v = ((t - m) ** 2).mean(-1, keepdims=True)
        return (t - m) / np.sqrt(v + EPS) * g + b

    x = x.astype(np.float32).copy()
    Bn, Sn, Dm = x.shape
    scale = 1.0 / np.sqrt(np.float32(DH))
    maskb = (mask == 0)[:, None]
    for i in range(L):
        xn1 = ln(x, ln1g[i], ln1b[i])
        xn2 = ln(x, ln2g[i], ln2b[i])

        def heads(t):
            return t.reshape(Bn, Sn, H, DH).transpose(0, 2, 1, 3)

        q = heads(xn1 @ wq[i] + bq[i])
        k = heads(xn1 @ wk[i] + bk[i])
        v = heads(xn1 @ wv[i] + bv[i])
        sc = np.einsum("bhqd,bhkd->bhqk", q, k) * scale
        sc = np.where(maskb, np.float32(-1e9), sc)
        sc = sc - sc.max(-1, keepdims=True)
        e = np.exp(sc)
        attn = e / e.sum(-1, keepdims=True)
        ctx = np.einsum("bhqk,bhkd->bhqd", attn, v)
        ctx = ctx.transpose(0, 2, 1, 3).reshape(Bn, Sn, Dm)
        x = x + (ctx @ wo[i] + bo[i])
        h = np.maximum(xn2 @ w1[i] + b1[i], 0.0)
        x = x + (h @ w2[i] + b2[i])
    return ln(x, lnfg, lnfb)


def kernel(**inputs):
    x = np.asarray(inputs["x"], dtype=np.float32)
    mask = np.asarray(inputs["mask"])
    if not (mask == 1).all():
        return _numpy_fallback(**{k: np.asarray(v) for k, v in inputs.items()})

    nc = _get_program()

    shared = {}
    for name in ("wq", "wk", "wv", "wo", "w1", "w2"):
        shared[name] = np.ascontiguousarray(np.asarray(inputs[name], dtype=np.float16))
    for name in ("bq", "bk", "bv", "bo", "b1", "b2",
                 "ln1g", "ln1b", "ln2g", "ln2b", "lnfg", "lnfb"):
        shared[name] = np.ascontiguousarray(np.asarray(inputs[name], dtype=np.float32))

    in_maps = []
    for c in range(NCORES):
        b, s = c // 2, c % 2
        xT = np.ascontiguousarray(x[b, s * T:(s + 1) * T, :].T)
        in_maps.append({"xT": xT, **shared})

    res = run_bass_kernel_spmd(nc, in_maps, list(range(NCORES)))

    out = np.empty((B, S, DM), dtype=np.float32)
    for c in range(NCORES):
        b, s = c // 2, c % 2
        out[b, s * T:(s + 1) * T, :] = res.results[c]["yT"].T
    return out
